# revision 1
# baseline (speedup 1.0000x reference)
"""Trainium2 Bass kernel for DiffuserAttention (GNN message passing).

v2 sharding: 8-way by dst-node range (1024 dst rows per core, full 768
feature width).  Edge scores are folded into diffusion step 1 via a
concatenated [k | v] gather table, so per-edge k rows ride the step-1
message gather for free.  Edge softmax is unnormalized (escale =
0.9*exp(score)); normalization (1/denom) is applied per-dst after each
segment-sum.  Per step: dma_gather h[src] rows (1536B/edge), DVE
broadcast-scale by escale, PE one-hot segment-sum into PSUM, per-dst
rdenom scale + 0.1*v teleport, 8-rank AllGather of the h shard (steps
1-4 only; step 5 output stays local for the output projection +
residual + LayerNorm).
"""

import sys

sys.path.insert(0, "/opt/trn_rl_repo")

import math

import numpy as np
import ml_dtypes

import concourse.bass as bass
import concourse.bacc as bacc
import concourse.mybir as mybir
import concourse.tile as tile
from concourse.bass_utils import run_bass_kernel_spmd

F32 = mybir.dt.float32
BF16 = mybir.dt.bfloat16
I16 = mybir.dt.int16
NPBF16 = ml_dtypes.bfloat16

NCORES = 8
NRANGE = 8
ALPHA = 0.1
NSTEPS = 5
LN_EPS = 1e-12
GCH = 8      # chunks per gather group (1024 idx = SWDGE ring capacity)


def _cfg(B, S, D, H, E):
    N = B * S
    cfg = dict(
        B=B, S=S, D=D, H=H, E=E, N=N,
        HD=D // H,
        NR=N // NRANGE,
    )
    cfg["NBLK"] = cfg["NR"] // 128
    cfg["DC"] = D // 128
    return cfg


def wrap_idx(idx):
    """dma_gather index layout: [128, n/16] int16; idx i at [i%16, i//16],
    replicated across the 8 Q7 cores."""
    n = idx.shape[0]
    w = idx.reshape(n // 16, 16).T.astype(np.int16)
    return np.ascontiguousarray(np.tile(w, (8, 1)))


def host_prep(cfg, hidden_states, attention_mask, src, dst,
              Wq, bq, Wk, bk, Wv, bv, Wo, bo, ln_g, ln_b):
    N, D, H, HD = cfg["N"], cfg["D"], cfg["H"], cfg["HD"]
    NR, NBLK = cfg["NR"], cfg["NBLK"]

    x = np.asarray(hidden_states, np.float32).reshape(N, D)
    src = np.asarray(src).astype(np.int64)
    dst = np.asarray(dst).astype(np.int64)
    mask1 = np.asarray(attention_mask).reshape(-1) >= 0
    all_valid = bool(mask1.all())

    # ---- edge partition by dst range, sort by dst, pad per 128-dst block
    per_range = []
    maxchunks = 0
    for r in range(NRANGE):
        sel = np.nonzero((dst >= r * NR) & (dst < (r + 1) * NR))[0]
        dl = dst[sel] - r * NR
        order = np.argsort(dl, kind="stable")
        sel = sel[order]
        dl = dl[order]
        counts = np.bincount(dl >> 7, minlength=NBLK)
        maxchunks = max(maxchunks, int(np.ceil(counts / 128).max()))
        per_range.append((sel, dl, counts))

    C_BLK = maxchunks
    while (NBLK * C_BLK) % GCH:
        C_BLK += 1
    NCHUNK = NBLK * C_BLK
    EP = NCHUNK * 128

    edges = []
    for r in range(NRANGE):
        sel, dl, counts = per_range[r]
        src_e = np.zeros(EP, np.int16)
        dstq_e = np.zeros(EP, np.int16)
        dstloc_e = np.zeros(EP, np.float32)
        valid_e = np.zeros(EP, np.float32)
        starts = np.concatenate([[0], np.cumsum(counts)])
        for b in range(NBLK):
            s0, s1 = starts[b], starts[b + 1]
            n = s1 - s0
            o = b * C_BLK * 128
            src_e[o:o + n] = src[sel[s0:s1]]
            dstq_e[o:o + n] = dl[s0:s1]
            dstloc_e[o:o + n] = (dl[s0:s1] - b * 128).astype(np.float32)
            if all_valid:
                valid_e[o:o + n] = 1.0
            else:
                valid_e[o:o + n] = (
                    mask1[src[sel[s0:s1]]] & mask1[dst[sel[s0:s1]]]
                ).astype(np.float32)
        edges.append(dict(
            src16=wrap_idx(src_e),
            dstq16=wrap_idx(dstq_e),
            dstloc=np.ascontiguousarray(dstloc_e.reshape(NCHUNK, 128).T),
            valid=np.ascontiguousarray(
                valid_e.reshape(NCHUNK, 128).T.astype(NPBF16)),
        ))

    # ---- weights / constants
    xT = np.ascontiguousarray(x.T.astype(NPBF16))          # [D, N]
    scale_q = 1.0 / math.sqrt(HD)
    Wq_s = np.ascontiguousarray((np.asarray(Wq) * scale_q).astype(NPBF16))
    Wk_s = np.ascontiguousarray(np.asarray(Wk).astype(NPBF16))
    Wv_s = np.ascontiguousarray(np.asarray(Wv).astype(NPBF16))
    bias3 = np.zeros((128, 3, D), np.float32)
    bias3[:, 0, :] = np.asarray(bq) * scale_q
    bias3[:, 1, :] = bk
    bias3[:, 2, :] = bv
    Wo_bf = np.ascontiguousarray(np.asarray(Wo).astype(NPBF16))

    iota = np.ascontiguousarray(
        np.broadcast_to(np.arange(128, dtype=np.float32), (128, 128))
    ).astype(NPBF16)
    ident = np.eye(128, dtype=np.float32).astype(NPBF16)
    g_rep = np.ascontiguousarray(
        np.broadcast_to(np.asarray(ln_g, np.float32), (128, D)))
    b_rep = np.ascontiguousarray(
        np.broadcast_to(np.asarray(ln_b, np.float32), (128, D)))

    in_maps = []
    for c in range(NCORES):
        rows = slice(c * NR, (c + 1) * NR)
        xTown = np.ascontiguousarray(xT[:, rows])
        xb = np.ascontiguousarray(x[rows] + np.asarray(bo, np.float32))
        m = dict(
            xT=xT, xTown=xTown, Wq=Wq_s, Wk=Wk_s, Wv=Wv_s,
            bias3=bias3, Wo=Wo_bf, xb=xb, g_rep=g_rep, b_rep=b_rep,
            iota=iota, ident=ident,
            **edges[c],
        )
        in_maps.append(m)
    zero_bias = not (np.any(bias3) or False)
    meta = dict(C_BLK=C_BLK, NCHUNK=NCHUNK, EP=EP, zero_bias=zero_bias)
    return in_maps, meta


def build_program(cfg, C_BLK, zero_bias=False):
    N, D, H, HD = cfg["N"], cfg["D"], cfg["H"], cfg["HD"]
    NR, NBLK, DC = cfg["NR"], cfg["NBLK"], cfg["DC"]
    NCHUNK = NBLK * C_BLK
    EP = NCHUNK * 128
    NT = N // 128
    NTO = NR // 128

    nc = bacc.Bacc(None, target_bir_lowering=False, debug=False,
                   num_devices=NCORES)

    xT_in = nc.dram_tensor("xT", [D, N], BF16, kind="ExternalInput")
    xTown_in = nc.dram_tensor("xTown", [D, NR], BF16, kind="ExternalInput")
    Wq_in = nc.dram_tensor("Wq", [D, D], BF16, kind="ExternalInput")
    Wk_in = nc.dram_tensor("Wk", [D, D], BF16, kind="ExternalInput")
    Wv_in = nc.dram_tensor("Wv", [D, D], BF16, kind="ExternalInput")
    bias3_in = nc.dram_tensor("bias3", [128, 3, D], F32, kind="ExternalInput")
    Wo_in = nc.dram_tensor("Wo", [D, D], BF16, kind="ExternalInput")
    xb_in = nc.dram_tensor("xb", [NR, D], F32, kind="ExternalInput")
    g_rep_in = nc.dram_tensor("g_rep", [128, D], F32, kind="ExternalInput")
    b_rep_in = nc.dram_tensor("b_rep", [128, D], F32, kind="ExternalInput")
    iota_in = nc.dram_tensor("iota", [128, 128], BF16, kind="ExternalInput")
    ident_in = nc.dram_tensor("ident", [128, 128], BF16, kind="ExternalInput")
    src16_in = nc.dram_tensor("src16", [128, EP // 16], I16, kind="ExternalInput")
    dstq16_in = nc.dram_tensor("dstq16", [128, EP // 16], I16, kind="ExternalInput")
    dstloc_in = nc.dram_tensor("dstloc", [128, NCHUNK], F32, kind="ExternalInput")
    valid_in = nc.dram_tensor("valid", [128, NCHUNK], BF16, kind="ExternalInput")

    out_ext = nc.dram_tensor("out", [NR, D], F32, kind="ExternalOutput")

    AG = [list(range(NCORES))]

    with tile.TileContext(nc) as tc:
        with (
            tc.tile_pool(name="res", bufs=1) as res,
            tc.tile_pool(name="dram", bufs=1, space="DRAM") as dram,
        ):
            iota_sb = res.tile([128, 128], BF16)
            nc.sync.dma_start(iota_sb[:], iota_in[:])
            ident_sb = res.tile([128, 128], BF16)
            nc.sync.dma_start(ident_sb[:], ident_in[:])
            ln09_sb = res.tile([128, 1], F32)
            nc.vector.memset(ln09_sb[:], float(np.log(0.9)))
            src16_sb = res.tile([128, EP // 16], I16)
            nc.sync.dma_start(src16_sb[:], src16_in[:])
            dstloc_sb = res.tile([128, NCHUNK], F32)
            nc.sync.dma_start(dstloc_sb[:], dstloc_in[:])
            escale_sb = res.tile([128, NCHUNK, H, 2], BF16)
            v01_sb = res.tile([128, NTO, D], BF16)
            h5_sb = res.tile([128, NTO, D], BF16)
            rdenom_sb = res.tile([128, NBLK, H], F32)

            q_dram = dram.tile([NR, D], BF16)
            kv_dram = dram.tile([N, 2 * D], BF16)
            hA = dram.tile([N, D], BF16)
            hB = dram.tile([N, D], BF16)
            shard = dram.tile([NR, D], BF16)

            # =========== P0: projections ===========
            with (
                tc.tile_pool(name="p0", bufs=3) as p0,
                tc.tile_pool(name="p0c", bufs=1) as p0c,
                tc.tile_pool(name="p0ps", bufs=4, space="PSUM") as p0ps,
            ):
                xTo_sb = p0c.tile([128, DC, NR], BF16)
                nc.sync.dma_start(
                    xTo_sb[:], xTown_in[:].rearrange("(c p) n -> p c n", p=128))
                W_sb = p0c.tile([128, 3, DC, D], BF16)
                for i, W in enumerate([Wq_in, Wk_in, Wv_in]):
                    nc.sync.dma_start(
                        W_sb[:, i, :, :],
                        W[:].rearrange("(c p) g -> p c g", p=128))
                bias_sb = p0c.tile([128, 3, D], F32)
                nc.sync.dma_start(bias_sb[:], bias3_in[:])

                def proj_tile(xsrc, toff, wi, stores, v01_t=None):
                    ps = p0ps.tile([128, D], F32, tag="pps")
                    for c in range(DC):
                        for j in range(2):
                            js = slice(j * 512, min((j + 1) * 512, D))
                            nc.tensor.matmul(
                                ps[:, js], xsrc[:, c, toff:toff + 128],
                                W_sb[:, wi, c, js],
                                start=(c == 0), stop=(c == DC - 1))
                    stg = p0.tile([128, D], BF16, tag="pstg")
                    if zero_bias:
                        nc.scalar.copy(stg[:], ps[:])
                    else:
                        nc.vector.tensor_tensor(
                            stg[:], ps[:], bias_sb[:, wi, :],
                            mybir.AluOpType.add)
                    for dest in stores:
                        nc.sync.dma_start(dest, stg[:])
                    if v01_t is not None:
                        nc.vector.tensor_scalar(
                            v01_sb[:, v01_t, :], stg[:], ALPHA, None,
                            mybir.AluOpType.mult)

                for t in range(NTO):
                    proj_tile(xTo_sb, t * 128, 0,
                              [q_dram[t * 128:(t + 1) * 128, :]])
                NH2 = N // 2
                for half in range(2):
                    xT_sb = p0c.tile([128, DC, NH2], BF16, tag="xTh")
                    nc.sync.dma_start(
                        xT_sb[:],
                        xT_in[:, half * NH2:(half + 1) * NH2].rearrange(
                            "(c p) n -> p c n", p=128))
                    for t in range(NT // 2):
                        tg = half * (NT // 2) + t
                        r = slice(tg * 128, (tg + 1) * 128)
                        proj_tile(xT_sb, t * 128, 1, [kv_dram[r, 0:D]])
                    for t in range(NT // 2):
                        tg = half * (NT // 2) + t
                        r = slice(tg * 128, (tg + 1) * 128)
                        proj_tile(xT_sb, t * 128, 2,
                                  [kv_dram[r, D:2 * D], hA[r, :]])
                for t in range(NTO):
                    proj_tile(xTo_sb, t * 128, 2, [], v01_t=t)

            # =========== P1+P2: scores (step 0) + diffusion ===========
            with (
                tc.tile_pool(name="p2kv", bufs=2) as p2kv,
                tc.tile_pool(name="p2h", bufs=3) as p2h,
                tc.tile_pool(name="p2q", bufs=2) as p2q,
                tc.tile_pool(name="p2o", bufs=6) as p2o,
                tc.tile_pool(name="p2s", bufs=2) as p2s,
                tc.tile_pool(name="p2e", bufs=1) as p2e,
                tc.tile_pool(name="p2ps", bufs=3, space="PSUM") as p2ps,
                tc.tile_pool(name="p2psd", bufs=2, space="PSUM") as p2psd,
            ):
                dstq16_sb = p2e.tile([128, EP // 16], I16)
                nc.sync.dma_start(dstq16_sb[:], dstq16_in[:])
                valid_sb = p2e.tile([128, NCHUNK], BF16)
                nc.sync.dma_start(valid_sb[:], valid_in[:])

                hsrcs = [kv_dram, hB, hA, hB, hA]
                hdsts = [hB, hA, hB, hA, None]
                for s in range(NSTEPS):
                    G = None
                    vG = None
                    psm = psd = None
                    oh = None
                    for q in range(NCHUNK):
                        blk, ch = divmod(q, C_BLK)
                        gc = q % GCH
                        if gc == 0:
                            ic = q * 8
                            gs = slice(q, q + GCH)
                            if s == 0:
                                G = p2kv.tile([128, GCH, 2 * D], BF16,
                                              tag="Gkv")
                                nc.gpsimd.dma_gather(
                                    G[:], kv_dram[:],
                                    src16_sb[:, ic:ic + GCH * 8],
                                    GCH * 128, GCH * 128, 2 * D)
                                Qg = p2q.tile([128, GCH, D], BF16, tag="Qg")
                                nc.gpsimd.dma_gather(
                                    Qg[:], q_dram[:],
                                    dstq16_sb[:, ic:ic + GCH * 8],
                                    GCH * 128, GCH * 128, D)
                                vG = G[:, :, D:2 * D]
                                nc.vector.tensor_tensor(
                                    Qg[:], G[:, :, 0:D], Qg[:],
                                    mybir.AluOpType.mult)
                                sc = p2s.tile([128, GCH, H], F32, tag="sc")
                                nc.vector.tensor_reduce(
                                    sc[:],
                                    Qg[:].rearrange(
                                        "p c (h f) -> p c h f", h=H),
                                    mybir.AxisListType.X,
                                    mybir.AluOpType.add)
                                nc.scalar.activation(
                                    escale_sb[:, gs, :, :],
                                    sc[:].unsqueeze(3).to_broadcast(
                                        (128, GCH, H, 2)),
                                    mybir.ActivationFunctionType.Exp,
                                    bias=ln09_sb[:], scale=1.0)
                                nc.vector.tensor_tensor(
                                    escale_sb[:, gs, :, :],
                                    escale_sb[:, gs, :, :],
                                    valid_sb[:, gs].unsqueeze(2)
                                    .to_broadcast((128, GCH, H))
                                    .unsqueeze(3)
                                    .to_broadcast((128, GCH, H, 2)),
                                    mybir.AluOpType.mult)
                            else:
                                G = p2h.tile([128, GCH, D], BF16, tag="Gh")
                                nc.gpsimd.dma_gather(
                                    G[:], hsrcs[s][:],
                                    src16_sb[:, ic:ic + GCH * 8],
                                    GCH * 128, GCH * 128, D)
                                vG = G[:, :, :]
                            oh = p2o.tile([128, GCH, 128], BF16, tag="oh")
                            nc.vector.tensor_tensor(
                                oh[:],
                                iota_sb[:].unsqueeze(1).to_broadcast(
                                    (128, GCH, 128)),
                                dstloc_sb[:, gs].unsqueeze(2).to_broadcast(
                                    (128, GCH, 128)),
                                mybir.AluOpType.is_equal)
                            nc.vector.tensor_tensor(
                                vG.rearrange(
                                    "p c (h f2 two) -> p c h f2 two",
                                    h=H, two=2),
                                vG.rearrange(
                                    "p c (h f2 two) -> p c h f2 two",
                                    h=H, two=2),
                                escale_sb[:, gs, :, :].unsqueeze(3)
                                .to_broadcast(
                                    (128, GCH, H, HD // 2, 2)),
                                mybir.AluOpType.mult)
                        if ch == 0:
                            psm = p2ps.tile([128, D], F32, tag="psm")
                            if s == 0:
                                psd = p2psd.tile([128, H], F32, tag="psd")
                        voff = D if s == 0 else 0
                        for j in range(2):
                            js = slice(voff + j * 512,
                                       voff + min((j + 1) * 512, D))
                            os_ = slice(j * 512, min((j + 1) * 512, D))
                            nc.tensor.matmul(
                                psm[:, os_], oh[:, gc, :], G[:, gc, js],
                                start=(ch == 0), stop=(ch == C_BLK - 1))
                        if s == 0:
                            nc.tensor.matmul(
                                psd[:], oh[:, gc, :],
                                escale_sb[:, q, :, 0:1].rearrange(
                                    "p h one -> p (h one)"),
                                start=(ch == 0), stop=(ch == C_BLK - 1))
                        if ch == C_BLK - 1:
                            if s == 0:
                                dn = p2s.tile([128, H], F32, tag="dn")
                                nc.vector.tensor_scalar(
                                    dn[:], psd[:], 0.9e-9, None,
                                    mybir.AluOpType.max)
                                dn2 = p2s.tile([128, H], F32, tag="dn2")
                                nc.vector.reciprocal(dn2[:], dn[:])
                                nc.vector.tensor_scalar(
                                    rdenom_sb[:, blk, :], dn2[:], 0.9, None,
                                    mybir.AluOpType.mult)
                            if s == NSTEPS - 1:
                                stg_ap = h5_sb[:, blk, :]
                            else:
                                stg = p2s.tile([128, D], BF16, tag="hstg")
                                stg_ap = stg[:]
                            for h in range(H):
                                hs = slice(h * HD, (h + 1) * HD)
                                nc.vector.scalar_tensor_tensor(
                                    stg_ap[:, hs], psm[:, hs],
                                    rdenom_sb[:, blk, h:h + 1],
                                    v01_sb[:, blk, hs],
                                    mybir.AluOpType.mult, mybir.AluOpType.add)
                            if s < NSTEPS - 1:
                                nc.sync.dma_start(
                                    shard[blk * 128:(blk + 1) * 128, :],
                                    stg_ap)
                    if s < NSTEPS - 1:
                        nc.gpsimd.collective_compute(
                            "AllGather", mybir.AluOpType.bypass,
                            replica_groups=AG,
                            ins=[shard[:].opt()], outs=[hdsts[s][:].opt()])

            # =========== P3: output projection + LN ===========
            with (
                tc.tile_pool(name="p3", bufs=2) as p3,
                tc.tile_pool(name="p3c", bufs=1) as p3c,
                tc.tile_pool(name="p3ps", bufs=4, space="PSUM") as p3ps,
                tc.tile_pool(name="p3ps2", bufs=2, space="PSUM") as p3ps2,
            ):
                g_sb = p3c.tile([128, D], F32)
                nc.sync.dma_start(g_sb[:], g_rep_in[:])
                b_sb = p3c.tile([128, D], F32)
                nc.sync.dma_start(b_sb[:], b_rep_in[:])
                h5T_sb = p3c.tile([128, DC, NR], BF16)
                for t in range(NTO):
                    for c in range(DC):
                        tp = p3ps.tile([128, 128], BF16, tag="tp")
                        nc.tensor.transpose(
                            tp[:], h5_sb[:, t, c * 128:(c + 1) * 128],
                            ident_sb[:])
                        nc.vector.tensor_copy(
                            h5T_sb[:, c, t * 128:(t + 1) * 128], tp[:])
                Wo_sb = p3c.tile([128, DC, D], BF16)
                nc.sync.dma_start(
                    Wo_sb[:], Wo_in[:].rearrange("(c p) n -> p c n", p=128))
                for t in range(NTO):
                    yps = p3ps2.tile([128, D], F32, tag="yps")
                    for c in range(DC):
                        for j in range(2):
                            js = slice(j * 512, min((j + 1) * 512, D))
                            nc.tensor.matmul(
                                yps[:, js],
                                h5T_sb[:, c, t * 128:(t + 1) * 128],
                                Wo_sb[:, c, js],
                                start=(c == 0), stop=(c == DC - 1))
                    xb_sb = p3.tile([128, D], F32, tag="xb")
                    nc.sync.dma_start(xb_sb[:], xb_in[t * 128:(t + 1) * 128, :])
                    y_sb = p3.tile([128, D], F32, tag="y")
                    nc.vector.tensor_tensor(
                        y_sb[:], yps[:], xb_sb[:], mybir.AluOpType.add)
                    mu = p3.tile([128, 1], F32, tag="mu")
                    nc.vector.tensor_reduce(
                        mu[:], y_sb[:], mybir.AxisListType.X,
                        mybir.AluOpType.add)
                    negmu = p3.tile([128, 1], F32, tag="negmu")
                    nc.vector.tensor_scalar(
                        negmu[:], mu[:], -1.0 / D, None, mybir.AluOpType.mult)
                    sq = p3.tile([128, D], F32, tag="sq")
                    var = p3.tile([128, 1], F32, tag="var")
                    nc.scalar.activation(
                        sq[:], y_sb[:], mybir.ActivationFunctionType.Square,
                        bias=negmu[:], scale=1.0, accum_out=var[:])
                    vs = p3.tile([128, 1], F32, tag="vs")
                    nc.vector.tensor_scalar(
                        vs[:], var[:], 1.0 / D, LN_EPS,
                        mybir.AluOpType.mult, mybir.AluOpType.add)
                    std = p3.tile([128, 1], F32, tag="std")
                    nc.scalar.sqrt(std[:], vs[:])
                    rstd = p3.tile([128, 1], F32, tag="rstd")
                    nc.vector.reciprocal(rstd[:], std[:])
                    t1 = p3.tile([128, D], F32, tag="t1")
                    nc.vector.scalar_tensor_tensor(
                        t1[:], y_sb[:], negmu[:], g_sb[:],
                        mybir.AluOpType.add, mybir.AluOpType.mult)
                    outt = p3.tile([128, D], F32, tag="outt")
                    nc.vector.scalar_tensor_tensor(
                        outt[:], t1[:], rstd[:], b_sb[:],
                        mybir.AluOpType.mult, mybir.AluOpType.add)
                    nc.sync.dma_start(
                        out_ext[t * 128:(t + 1) * 128, :], outt[:])

    nc.compile()
    return nc


_PROG_CACHE = {}


def _get_program(cfg, C_BLK, zero_bias):
    key = (cfg["N"], cfg["E"], cfg["D"], cfg["H"], C_BLK, zero_bias)
    if key not in _PROG_CACHE:
        _PROG_CACHE[key] = build_program(cfg, C_BLK, zero_bias)
    return _PROG_CACHE[key]


def run(cfg, inputs, trace=False):
    in_maps, meta = host_prep(cfg, **inputs)
    nc = _get_program(cfg, meta["C_BLK"], meta["zero_bias"])
    res = run_bass_kernel_spmd(
        nc, in_maps, core_ids=list(range(NCORES)), trace=trace)
    N, D, NR = cfg["N"], cfg["D"], cfg["NR"]
    full = np.empty((N, D), np.float32)
    for r in range(NRANGE):
        full[r * NR:(r + 1) * NR] = res.results[r]["out"]
    return full.reshape(cfg["B"], cfg["S"], D), res


def kernel(**inputs):
    cfg = _cfg(B=2, S=4096, D=768, H=12, E=524288)
    out, _ = run(cfg, inputs)
    return out



# revision 5
# speedup vs baseline: 1.0381x; 1.0381x over previous
"""Trainium2 Bass kernel for DiffuserAttention (GNN message passing).

v3 sharding: 8-way by dst-node range (1024 dst rows per core, full 768
feature width).  Edge scores are folded into diffusion step 1 via a
concatenated [k | v] gather table.  Scatter one-hots (oh) and their
transposes (ohT) are host-precomputed fp8 matrices streamed from DRAM:
oh is the stationary operand of the segment-sum matmul, ohT selects
q[dst] rows from the local 128-dst q block at step 0 (replacing the
per-edge q dma_gather -- SWDGE descriptor generation on GPSIMD is the
kernel bottleneck at ~9ns/row).  Edge softmax is unnormalized
(escale = 7.2*exp(score)); normalization (0.9/denom) is applied
per-dst after each segment-sum, which cancels the 7.2.  Per step:
dma_gather h[src] rows, DVE broadcast-scale by escale, PE one-hot
segment-sum into PSUM, per-dst rdenom scale + 0.1*v teleport, 8-rank
AllGather of the h shard into a Shared scratchpad table (steps 1-4;
step 5 output stays local for output projection + residual + LN).
"""

import sys

sys.path.insert(0, "/opt/trn_rl_repo")

import math

import numpy as np
import ml_dtypes

import concourse.bass as bass
import concourse.bacc as bacc
import concourse.mybir as mybir
import concourse.tile as tile
from concourse.bass_utils import run_bass_kernel_spmd

F32 = mybir.dt.float32
BF16 = mybir.dt.bfloat16
FP8 = mybir.dt.float8e4
I16 = mybir.dt.int16
NPBF16 = ml_dtypes.bfloat16
NPFP8 = ml_dtypes.float8_e4m3

NCORES = 8
NRANGE = 8
ALPHA = 0.1
NSTEPS = 5
LN_EPS = 1e-12
GCH = 8      # chunks per gather group (1024 idx = SWDGE ring capacity)
KAPPA = 8.0  # escale prescale (cancelled by rdenom); keeps msg in range


def _cfg(B, S, D, H, E):
    N = B * S
    cfg = dict(
        B=B, S=S, D=D, H=H, E=E, N=N,
        HD=D // H,
        NR=N // NRANGE,
    )
    cfg["NBLK"] = cfg["NR"] // 128
    cfg["DC"] = D // 128
    return cfg


def wrap_idx(idx):
    """dma_gather index layout: [128, n/16] int16; idx i at [i%16, i//16],
    replicated across the 8 Q7 cores."""
    n = idx.shape[0]
    w = idx.reshape(n // 16, 16).T.astype(np.int16)
    return np.ascontiguousarray(np.tile(w, (8, 1)))


def host_prep(cfg, hidden_states, attention_mask, src, dst,
              Wq, bq, Wk, bk, Wv, bv, Wo, bo, ln_g, ln_b):
    N, D, H, HD = cfg["N"], cfg["D"], cfg["H"], cfg["HD"]
    NR, NBLK = cfg["NR"], cfg["NBLK"]

    x = np.asarray(hidden_states, np.float32).reshape(N, D)
    src = np.asarray(src).astype(np.int64)
    dst = np.asarray(dst).astype(np.int64)
    mask1 = np.asarray(attention_mask).reshape(-1) >= 0
    all_valid = bool(mask1.all())

    # ---- edge partition by dst range, then by 128-dst block; within a
    # block sort by src (better HBM locality for the row gathers)
    per_range = []
    maxchunks = 0
    for r in range(NRANGE):
        sel = np.nonzero((dst >= r * NR) & (dst < (r + 1) * NR))[0]
        dl = dst[sel] - r * NR
        order = np.lexsort((src[sel], dl >> 7))
        sel = sel[order]
        dl = dl[order]
        counts = np.bincount(dl >> 7, minlength=NBLK)
        maxchunks = max(maxchunks, int(np.ceil(counts / 128).max()))
        per_range.append((sel, dl, counts))

    C_BLK = maxchunks
    while (NBLK * C_BLK) % GCH:
        C_BLK += 1
    NCHUNK = NBLK * C_BLK
    EP = NCHUNK * 128

    edges = []
    for r in range(NRANGE):
        sel, dl, counts = per_range[r]
        src_e = np.zeros(EP, np.int16)
        dstloc_e = np.zeros(EP, np.int64)
        live_e = np.zeros(EP, bool)
        valid_e = np.zeros(EP, np.float32)
        starts = np.concatenate([[0], np.cumsum(counts)])
        for b in range(NBLK):
            s0, s1 = starts[b], starts[b + 1]
            n = s1 - s0
            o = b * C_BLK * 128
            src_e[o:o + n] = src[sel[s0:s1]]
            dstloc_e[o:o + n] = dl[s0:s1] - b * 128
            live_e[o:o + n] = True
            if all_valid:
                valid_e[o:o + n] = 1.0
            else:
                valid_e[o:o + n] = (
                    mask1[src[sel[s0:s1]]] & mask1[dst[sel[s0:s1]]]
                ).astype(np.float32)

        # one-hot scatter matrices: oh[e, d] = live(e) & (dstloc(e)==d),
        # stored partition-major [128 e, NCHUNK, 128 d] for contiguous DMA
        oh = np.zeros(EP * 128, NPFP8)
        ee = np.nonzero(live_e)[0]
        oh[ee * 128 + dstloc_e[ee]] = 1.0
        oh = oh.reshape(NCHUNK, 128, 128)
        ohT = np.zeros((NCHUNK, 128, 128), NPFP8)
        cid = ee >> 7
        epos = ee & 127
        ohT[cid, dstloc_e[ee], epos] = 1.0
        edges.append(dict(
            src16=wrap_idx(src_e),
            oh=np.ascontiguousarray(oh.transpose(1, 0, 2)),
            ohT=np.ascontiguousarray(ohT.transpose(1, 0, 2)),
            valid=np.ascontiguousarray(
                valid_e.reshape(NCHUNK, 128).T.astype(NPBF16)),
        ))

    # ---- weights / constants
    xT = np.ascontiguousarray(x.T.astype(NPBF16))          # [D, N]
    scale_q = 1.0 / math.sqrt(HD)
    Wq_s = np.ascontiguousarray((np.asarray(Wq) * scale_q).astype(NPBF16))
    Wk_s = np.ascontiguousarray(np.asarray(Wk).astype(NPBF16))
    Wv_s = np.ascontiguousarray(np.asarray(Wv).astype(NPBF16))
    bias3 = np.zeros((128, 3, D), np.float32)
    bias3[:, 0, :] = np.asarray(bq) * scale_q
    bias3[:, 1, :] = bk
    bias3[:, 2, :] = bv
    Wo_bf = np.ascontiguousarray(np.asarray(Wo).astype(NPBF16))

    ident = np.eye(128, dtype=np.float32).astype(NPBF16)
    g_rep = np.ascontiguousarray(
        np.broadcast_to(np.asarray(ln_g, np.float32), (128, D)))
    b_rep = np.ascontiguousarray(
        np.broadcast_to(np.asarray(ln_b, np.float32), (128, D)))

    in_maps = []
    for c in range(NCORES):
        rows = slice(c * NR, (c + 1) * NR)
        xTown = np.ascontiguousarray(xT[:, rows])
        xb = np.ascontiguousarray(x[rows] + np.asarray(bo, np.float32))
        m = dict(
            xT=xT, xTown=xTown, Wq=Wq_s, Wk=Wk_s, Wv=Wv_s,
            bias3=bias3, Wo=Wo_bf, xb=xb, g_rep=g_rep, b_rep=b_rep,
            ident=ident,
            **edges[c],
        )
        in_maps.append(m)
    zero_bias = not (np.any(bias3) or False)
    meta = dict(C_BLK=C_BLK, NCHUNK=NCHUNK, EP=EP, zero_bias=zero_bias)
    return in_maps, meta


def build_program(cfg, C_BLK, zero_bias=False):
    N, D, H, HD = cfg["N"], cfg["D"], cfg["H"], cfg["HD"]
    NR, NBLK, DC = cfg["NR"], cfg["NBLK"], cfg["DC"]
    NCHUNK = NBLK * C_BLK
    EP = NCHUNK * 128
    NT = N // 128
    NTO = NR // 128

    nc = bacc.Bacc(None, target_bir_lowering=False, debug=False,
                   num_devices=NCORES)

    xT_in = nc.dram_tensor("xT", [D, N], BF16, kind="ExternalInput")
    xTown_in = nc.dram_tensor("xTown", [D, NR], BF16, kind="ExternalInput")
    Wq_in = nc.dram_tensor("Wq", [D, D], BF16, kind="ExternalInput")
    Wk_in = nc.dram_tensor("Wk", [D, D], BF16, kind="ExternalInput")
    Wv_in = nc.dram_tensor("Wv", [D, D], BF16, kind="ExternalInput")
    bias3_in = nc.dram_tensor("bias3", [128, 3, D], F32, kind="ExternalInput")
    Wo_in = nc.dram_tensor("Wo", [D, D], BF16, kind="ExternalInput")
    xb_in = nc.dram_tensor("xb", [NR, D], F32, kind="ExternalInput")
    g_rep_in = nc.dram_tensor("g_rep", [128, D], F32, kind="ExternalInput")
    b_rep_in = nc.dram_tensor("b_rep", [128, D], F32, kind="ExternalInput")
    ident_in = nc.dram_tensor("ident", [128, 128], BF16, kind="ExternalInput")
    src16_in = nc.dram_tensor("src16", [128, EP // 16], I16, kind="ExternalInput")
    oh_in = nc.dram_tensor("oh", [128, NCHUNK, 128], FP8, kind="ExternalInput")
    ohT_in = nc.dram_tensor("ohT", [128, NCHUNK, 128], FP8, kind="ExternalInput")
    valid_in = nc.dram_tensor("valid", [128, NCHUNK], BF16, kind="ExternalInput")

    out_ext = nc.dram_tensor("out", [NR, D], F32, kind="ExternalOutput")

    # AllGather destinations: Shared scratchpad for fast HBM-HBM collective
    hA = nc.dram_tensor("hAsh", [N, D], BF16, kind="Internal",
                        addr_space="Shared")
    hB = nc.dram_tensor("hBsh", [N, D], BF16, kind="Internal",
                        addr_space="Shared")

    AG = [list(range(NCORES))]

    with tile.TileContext(nc) as tc:
        with (
            tc.tile_pool(name="res", bufs=1) as res,
            tc.tile_pool(name="dram", bufs=1, space="DRAM") as dram,
        ):
            ident_sb = res.tile([128, 128], BF16)
            nc.sync.dma_start(ident_sb[:], ident_in[:])
            ln72_sb = res.tile([128, 1], F32)
            nc.vector.memset(ln72_sb[:], float(np.log(0.9 * KAPPA)))
            src16_sb = res.tile([128, EP // 16], I16)
            nc.sync.dma_start(src16_sb[:], src16_in[:])
            escale_sb = res.tile([128, NCHUNK, H, 2], BF16)
            v01_sb = res.tile([128, NTO, D], BF16)
            h5_sb = res.tile([128, NTO, D], BF16)
            rdenom_sb = res.tile([128, NBLK, H], F32)

            q_dram = dram.tile([NR, D], BF16)
            kv_dram = dram.tile([N, 2 * D], BF16)
            shard = dram.tile([NR, D], BF16)

            # =========== P0: projections ===========
            with (
                tc.tile_pool(name="p0", bufs=3) as p0,
                tc.tile_pool(name="p0c", bufs=1) as p0c,
                tc.tile_pool(name="p0ps", bufs=4, space="PSUM") as p0ps,
            ):
                xTo_sb = p0c.tile([128, DC, NR], BF16)
                nc.sync.dma_start(
                    xTo_sb[:], xTown_in[:].rearrange("(c p) n -> p c n", p=128))
                W_sb = p0c.tile([128, 3, DC, D], BF16)
                for i, W in enumerate([Wq_in, Wk_in, Wv_in]):
                    nc.sync.dma_start(
                        W_sb[:, i, :, :],
                        W[:].rearrange("(c p) g -> p c g", p=128))
                bias_sb = p0c.tile([128, 3, D], F32)
                nc.sync.dma_start(bias_sb[:], bias3_in[:])

                def proj_tile(xsrc, toff, wi, stores, v01_t=None):
                    ps = p0ps.tile([128, D], F32, tag="pps")
                    for c in range(DC):
                        for j in range(2):
                            js = slice(j * 512, min((j + 1) * 512, D))
                            nc.tensor.matmul(
                                ps[:, js], xsrc[:, c, toff:toff + 128],
                                W_sb[:, wi, c, js],
                                start=(c == 0), stop=(c == DC - 1))
                    stg = p0.tile([128, D], BF16, tag="pstg")
                    if zero_bias:
                        nc.scalar.copy(stg[:], ps[:])
                    else:
                        nc.vector.tensor_tensor(
                            stg[:], ps[:], bias_sb[:, wi, :],
                            mybir.AluOpType.add)
                    for dest in stores:
                        nc.sync.dma_start(dest, stg[:])
                    if v01_t is not None:
                        nc.vector.tensor_scalar(
                            v01_sb[:, v01_t, :], stg[:], ALPHA, None,
                            mybir.AluOpType.mult)

                for t in range(NTO):
                    proj_tile(xTo_sb, t * 128, 0,
                              [q_dram[t * 128:(t + 1) * 128, :]])
                NH2 = N // 2
                for half in range(2):
                    xT_sb = p0c.tile([128, DC, NH2], BF16, tag="xTh")
                    nc.sync.dma_start(
                        xT_sb[:],
                        xT_in[:, half * NH2:(half + 1) * NH2].rearrange(
                            "(c p) n -> p c n", p=128))
                    for t in range(NT // 2):
                        tg = half * (NT // 2) + t
                        r = slice(tg * 128, (tg + 1) * 128)
                        proj_tile(xT_sb, t * 128, 1, [kv_dram[r, 0:D]])
                    for t in range(NT // 2):
                        tg = half * (NT // 2) + t
                        r = slice(tg * 128, (tg + 1) * 128)
                        proj_tile(xT_sb, t * 128, 2, [kv_dram[r, D:2 * D]])
                for t in range(NTO):
                    proj_tile(xTo_sb, t * 128, 2, [], v01_t=t)

            # =========== P1+P2: scores (step 0) + diffusion ===========
            with (
                tc.tile_pool(name="p2kv", bufs=2) as p2kv,
                tc.tile_pool(name="p2h", bufs=3) as p2h,
                tc.tile_pool(name="p2oh", bufs=3) as p2oh,
                tc.tile_pool(name="p2q", bufs=2) as p2q,
                tc.tile_pool(name="p2qb", bufs=2) as p2qb,
                tc.tile_pool(name="p2s", bufs=3) as p2s,
                tc.tile_pool(name="p2e", bufs=1) as p2e,
                tc.tile_pool(name="p2ps", bufs=2, space="PSUM") as p2ps,
                tc.tile_pool(name="p2psq", bufs=2, space="PSUM") as p2psq,
            ):
                valid_sb = p2e.tile([128, NCHUNK], BF16)
                nc.sync.dma_start(valid_sb[:], valid_in[:])

                hsrcs = [kv_dram, hB, hA, hB, hA]
                hdsts = [hB, hA, hB, hA, None]
                for s in range(NSTEPS):
                    G = None
                    vG = None
                    oh = None
                    psm = None
                    qblk = None
                    for q in range(NCHUNK):
                        blk, ch = divmod(q, C_BLK)
                        gc = q % GCH
                        if gc == 0:
                            ic = q * 8
                            gs = slice(q, q + GCH)
                            oh = p2oh.tile([128, GCH, 128], FP8, tag="oh")
                            nc.sync.dma_start(
                                oh[:], oh_in[:, gs, :])
                            if s == 0:
                                G = p2kv.tile([128, GCH, 2 * D], BF16,
                                              tag="Gkv")
                                nc.gpsimd.dma_gather(
                                    G[:], kv_dram[:],
                                    src16_sb[:, ic:ic + GCH * 8],
                                    GCH * 128, GCH * 128, 2 * D)
                                ohT = p2q.tile([128, GCH, 128], FP8,
                                               tag="ohT")
                                nc.sync.dma_start(
                                    ohT[:], ohT_in[:, gs, :])
                                vG = G[:, :, D:2 * D]
                            else:
                                G = p2h.tile([128, GCH, D], BF16, tag="Gh")
                                nc.gpsimd.dma_gather(
                                    G[:], hsrcs[s][:],
                                    src16_sb[:, ic:ic + GCH * 8],
                                    GCH * 128, GCH * 128, D)
                                vG = G[:, :, :]
                        if ch == 0:
                            psm = p2ps.tile([128, 832], F32, tag="psm")
                            if s == 0:
                                qblk = p2qb.tile([128, D], BF16, tag="qblk")
                                nc.sync.dma_start(
                                    qblk[:],
                                    q_dram[blk * 128:(blk + 1) * 128, :])
                        if s == 0:
                            # qsel[e,:] = q_blk[dstloc(e),:] via ohT matmul
                            qps = p2psq.tile([128, D], F32, tag="qps")
                            for j in range(2):
                                js = slice(j * 512, min((j + 1) * 512, D))
                                nc.tensor.matmul(
                                    qps[:, js], ohT[:, gc, :], qblk[:, js],
                                    start=True, stop=True)
                            qb16 = p2s.tile([128, D], BF16, tag="qb16")
                            nc.scalar.copy(qb16[:], qps[:])
                            prod = p2s.tile([128, H, HD], BF16, tag="prod")
                            nc.vector.tensor_tensor(
                                prod[:],
                                G[:, gc, 0:D].rearrange(
                                    "p (h f) -> p h f", h=H),
                                qb16[:].rearrange("p (h f) -> p h f", h=H),
                                mybir.AluOpType.mult)
                            # per-head tree reduce over HD=64
                            w = HD // 2
                            while w >= 1:
                                nc.vector.tensor_tensor(
                                    prod[:, :, 0:w], prod[:, :, 0:w],
                                    prod[:, :, w:2 * w],
                                    mybir.AluOpType.add)
                                w //= 2
                            nc.scalar.activation(
                                escale_sb[:, q, :, :],
                                prod[:, :, 0:1].to_broadcast((128, H, 2)),
                                mybir.ActivationFunctionType.Exp,
                                bias=ln72_sb[:], scale=1.0)
                            nc.vector.tensor_tensor(
                                escale_sb[:, q, :, :],
                                escale_sb[:, q, :, :],
                                valid_sb[:, q:q + 1].to_broadcast((128, H))
                                .unsqueeze(2).to_broadcast((128, H, 2)),
                                mybir.AluOpType.mult)
                            nc.vector.tensor_tensor(
                                vG[:, gc, :].rearrange(
                                    "p (h f2 two) -> p h f2 two",
                                    h=H, two=2),
                                vG[:, gc, :].rearrange(
                                    "p (h f2 two) -> p h f2 two",
                                    h=H, two=2),
                                escale_sb[:, q, :, :].unsqueeze(2)
                                .to_broadcast((128, H, HD // 2, 2)),
                                mybir.AluOpType.mult)
                        else:
                            nc.vector.tensor_tensor(
                                vG[:, gc, :].rearrange(
                                    "p (h f2 two) -> p h f2 two",
                                    h=H, two=2),
                                vG[:, gc, :].rearrange(
                                    "p (h f2 two) -> p h f2 two",
                                    h=H, two=2),
                                escale_sb[:, q, :, :].unsqueeze(2)
                                .to_broadcast((128, H, HD // 2, 2)),
                                mybir.AluOpType.mult)
                        voff = D if s == 0 else 0
                        for j in range(2):
                            js = slice(voff + j * 512,
                                       voff + min((j + 1) * 512, D))
                            os_ = slice(j * 512, min((j + 1) * 512, D))
                            nc.tensor.matmul(
                                psm[:, os_], oh[:, gc, :], G[:, gc, js],
                                start=(ch == 0), stop=(ch == C_BLK - 1))
                        if s == 0:
                            nc.tensor.matmul(
                                psm[:, 768:768 + H], oh[:, gc, :],
                                escale_sb[:, q, :, 0:1].rearrange(
                                    "p h one -> p (h one)"),
                                start=(ch == 0), stop=(ch == C_BLK - 1),
                                skip_group_check=True)
                        if ch == C_BLK - 1:
                            if s == 0:
                                dn = p2s.tile([128, H], F32, tag="dn")
                                nc.vector.tensor_scalar(
                                    dn[:], psm[:, 768:768 + H], 1e-9, None,
                                    mybir.AluOpType.max)
                                dn2 = p2s.tile([128, H], F32, tag="dn2")
                                nc.vector.reciprocal(dn2[:], dn[:])
                                nc.vector.tensor_scalar(
                                    rdenom_sb[:, blk, :], dn2[:], 0.9, None,
                                    mybir.AluOpType.mult)
                            if s == NSTEPS - 1:
                                stg_ap = h5_sb[:, blk, :]
                            else:
                                stg = p2s.tile([128, D], BF16, tag="hstg")
                                stg_ap = stg[:]
                            for h in range(H):
                                hs = slice(h * HD, (h + 1) * HD)
                                nc.vector.scalar_tensor_tensor(
                                    stg_ap[:, hs], psm[:, hs],
                                    rdenom_sb[:, blk, h:h + 1],
                                    v01_sb[:, blk, hs],
                                    mybir.AluOpType.mult, mybir.AluOpType.add)
                            if s < NSTEPS - 1:
                                nc.sync.dma_start(
                                    shard[blk * 128:(blk + 1) * 128, :],
                                    stg_ap)
                    if s < NSTEPS - 1:
                        nc.gpsimd.collective_compute(
                            "AllGather", mybir.AluOpType.bypass,
                            replica_groups=AG,
                            ins=[shard[:].opt()], outs=[hdsts[s][:].opt()])

            # =========== P3: output projection + LN ===========
            with (
                tc.tile_pool(name="p3", bufs=2) as p3,
                tc.tile_pool(name="p3c", bufs=1) as p3c,
                tc.tile_pool(name="p3ps", bufs=4, space="PSUM") as p3ps,
                tc.tile_pool(name="p3ps2", bufs=2, space="PSUM") as p3ps2,
            ):
                g_sb = p3c.tile([128, D], F32)
                nc.sync.dma_start(g_sb[:], g_rep_in[:])
                b_sb = p3c.tile([128, D], F32)
                nc.sync.dma_start(b_sb[:], b_rep_in[:])
                h5T_sb = p3c.tile([128, DC, NR], BF16)
                for t in range(NTO):
                    for c in range(DC):
                        tp = p3ps.tile([128, 128], BF16, tag="tp")
                        nc.tensor.transpose(
                            tp[:], h5_sb[:, t, c * 128:(c + 1) * 128],
                            ident_sb[:])
                        nc.vector.tensor_copy(
                            h5T_sb[:, c, t * 128:(t + 1) * 128], tp[:])
                Wo_sb = p3c.tile([128, DC, D], BF16)
                nc.sync.dma_start(
                    Wo_sb[:], Wo_in[:].rearrange("(c p) n -> p c n", p=128))
                for t in range(NTO):
                    yps = p3ps2.tile([128, D], F32, tag="yps")
                    for c in range(DC):
                        for j in range(2):
                            js = slice(j * 512, min((j + 1) * 512, D))
                            nc.tensor.matmul(
                                yps[:, js],
                                h5T_sb[:, c, t * 128:(t + 1) * 128],
                                Wo_sb[:, c, js],
                                start=(c == 0), stop=(c == DC - 1))
                    xb_sb = p3.tile([128, D], F32, tag="xb")
                    nc.sync.dma_start(xb_sb[:], xb_in[t * 128:(t + 1) * 128, :])
                    y_sb = p3.tile([128, D], F32, tag="y")
                    nc.vector.tensor_tensor(
                        y_sb[:], yps[:], xb_sb[:], mybir.AluOpType.add)
                    mu = p3.tile([128, 1], F32, tag="mu")
                    nc.vector.tensor_reduce(
                        mu[:], y_sb[:], mybir.AxisListType.X,
                        mybir.AluOpType.add)
                    negmu = p3.tile([128, 1], F32, tag="negmu")
                    nc.vector.tensor_scalar(
                        negmu[:], mu[:], -1.0 / D, None, mybir.AluOpType.mult)
                    sq = p3.tile([128, D], F32, tag="sq")
                    var = p3.tile([128, 1], F32, tag="var")
                    nc.scalar.activation(
                        sq[:], y_sb[:], mybir.ActivationFunctionType.Square,
                        bias=negmu[:], scale=1.0, accum_out=var[:])
                    vs = p3.tile([128, 1], F32, tag="vs")
                    nc.vector.tensor_scalar(
                        vs[:], var[:], 1.0 / D, LN_EPS,
                        mybir.AluOpType.mult, mybir.AluOpType.add)
                    std = p3.tile([128, 1], F32, tag="std")
                    nc.scalar.sqrt(std[:], vs[:])
                    rstd = p3.tile([128, 1], F32, tag="rstd")
                    nc.vector.reciprocal(rstd[:], std[:])
                    t1 = p3.tile([128, D], F32, tag="t1")
                    nc.vector.scalar_tensor_tensor(
                        t1[:], y_sb[:], negmu[:], g_sb[:],
                        mybir.AluOpType.add, mybir.AluOpType.mult)
                    outt = p3.tile([128, D], F32, tag="outt")
                    nc.vector.scalar_tensor_tensor(
                        outt[:], t1[:], rstd[:], b_sb[:],
                        mybir.AluOpType.mult, mybir.AluOpType.add)
                    nc.sync.dma_start(
                        out_ext[t * 128:(t + 1) * 128, :], outt[:])

    nc.compile()
    return nc


_PROG_CACHE = {}


def _get_program(cfg, C_BLK, zero_bias):
    key = (cfg["N"], cfg["E"], cfg["D"], cfg["H"], C_BLK, zero_bias)
    if key not in _PROG_CACHE:
        _PROG_CACHE[key] = build_program(cfg, C_BLK, zero_bias)
    return _PROG_CACHE[key]


def run(cfg, inputs, trace=False):
    in_maps, meta = host_prep(cfg, **inputs)
    nc = _get_program(cfg, meta["C_BLK"], meta["zero_bias"])
    res = run_bass_kernel_spmd(
        nc, in_maps, core_ids=list(range(NCORES)), trace=trace)
    N, D, NR = cfg["N"], cfg["D"], cfg["NR"]
    full = np.empty((N, D), np.float32)
    for r in range(NRANGE):
        full[r * NR:(r + 1) * NR] = res.results[r]["out"]
    return full.reshape(cfg["B"], cfg["S"], D), res


def kernel(**inputs):
    cfg = _cfg(B=2, S=4096, D=768, H=12, E=524288)
    out, _ = run(cfg, inputs)
    return out


# revision 13
# speedup vs baseline: 1.1094x; 1.0686x over previous
"""Trainium2 Bass kernel for DiffuserAttention (GNN message passing).

v3 sharding: 8-way by dst-node range (1024 dst rows per core, full 768
feature width).  Edge scores are folded into diffusion step 1 via a
concatenated [k | v] gather table.  Scatter one-hots (oh) and their
transposes (ohT) are host-precomputed fp8 matrices streamed from DRAM:
oh is the stationary operand of the segment-sum matmul, ohT selects
q[dst] rows from the local 128-dst q block at step 0 (replacing the
per-edge q dma_gather -- SWDGE descriptor generation on GPSIMD is the
kernel bottleneck at ~9ns/row).  Edge softmax is unnormalized
(escale = 7.2*exp(score)); normalization (0.9/denom) is applied
per-dst after each segment-sum, which cancels the 7.2.  Per step:
dma_gather h[src] rows, DVE broadcast-scale by escale, PE one-hot
segment-sum into PSUM, per-dst rdenom scale + 0.1*v teleport, 8-rank
AllGather of the h shard into a Shared scratchpad table (steps 1-4;
step 5 output stays local for output projection + residual + LN).
"""

import sys

sys.path.insert(0, "/opt/trn_rl_repo")

import math

import numpy as np
import ml_dtypes

import concourse.bass as bass
import concourse.bacc as bacc
import concourse.mybir as mybir
import concourse.tile as tile
from concourse.bass_utils import run_bass_kernel_spmd

F32 = mybir.dt.float32
BF16 = mybir.dt.bfloat16
FP8 = mybir.dt.float8e4
I16 = mybir.dt.int16
NPBF16 = ml_dtypes.bfloat16
NPFP8 = ml_dtypes.float8_e4m3

NCORES = 8
NRANGE = 8
ALPHA = 0.1
NSTEPS = 5
LN_EPS = 1e-12
GCH = 8      # chunks per gather group (1024 idx = SWDGE ring capacity)
KAPPA = 8.0  # escale prescale (cancelled by rdenom); keeps msg in range


def _cfg(B, S, D, H, E):
    N = B * S
    cfg = dict(
        B=B, S=S, D=D, H=H, E=E, N=N,
        HD=D // H,
        NR=N // NRANGE,
    )
    cfg["NBLK"] = cfg["NR"] // 128
    cfg["DC"] = D // 128
    return cfg


def wrap_idx(idx):
    """dma_gather index layout: [128, n/16] int16; idx i at [i%16, i//16],
    replicated across the 8 Q7 cores."""
    n = idx.shape[0]
    w = idx.reshape(n // 16, 16).T.astype(np.int16)
    return np.ascontiguousarray(np.tile(w, (8, 1)))


def host_prep(cfg, hidden_states, attention_mask, src, dst,
              Wq, bq, Wk, bk, Wv, bv, Wo, bo, ln_g, ln_b):
    N, D, H, HD = cfg["N"], cfg["D"], cfg["H"], cfg["HD"]
    NR, NBLK = cfg["NR"], cfg["NBLK"]

    x = np.asarray(hidden_states, np.float32).reshape(N, D)
    src = np.asarray(src).astype(np.int64)
    dst = np.asarray(dst).astype(np.int64)
    mask1 = np.asarray(attention_mask).reshape(-1) >= 0
    all_valid = bool(mask1.all())

    # ---- edge partition by dst range, then by 128-dst block; within a
    # block sort by src (better HBM locality for the row gathers)
    per_range = []
    maxchunks = 0
    for r in range(NRANGE):
        sel = np.nonzero((dst >= r * NR) & (dst < (r + 1) * NR))[0]
        dl = dst[sel] - r * NR
        order = np.lexsort((src[sel], dl >> 7))
        sel = sel[order]
        dl = dl[order]
        counts = np.bincount(dl >> 7, minlength=NBLK)
        maxchunks = max(maxchunks, int(np.ceil(counts / 128).max()))
        per_range.append((sel, dl, counts))

    C_BLK = maxchunks
    while (NBLK * C_BLK) % GCH:
        C_BLK += 1
    NCHUNK = NBLK * C_BLK
    EP = NCHUNK * 128

    edges = []
    for r in range(NRANGE):
        sel, dl, counts = per_range[r]
        src_e = np.zeros(EP, np.int16)
        dstloc_e = np.zeros(EP, np.int64)
        live_e = np.zeros(EP, bool)
        valid_e = np.zeros(EP, bool)
        starts = np.concatenate([[0], np.cumsum(counts)])
        for b in range(NBLK):
            s0, s1 = starts[b], starts[b + 1]
            n = s1 - s0
            o = b * C_BLK * 128
            src_e[o:o + n] = src[sel[s0:s1]]
            dstloc_e[o:o + n] = dl[s0:s1] - b * 128
            live_e[o:o + n] = True
            if all_valid:
                valid_e[o:o + n] = True
            else:
                valid_e[o:o + n] = (
                    mask1[src[sel[s0:s1]]] & mask1[dst[sel[s0:s1]]])

        # one-hot scatter matrices: oh[e, d] = valid(e) & (dstloc(e)==d)
        # (validity folded in: a zero row drops the edge from both the
        # numerator and the softmax denominator, as masking requires);
        # stored partition-major [128 e, NCHUNK, 128 d] for contiguous DMA
        oh = np.zeros(EP * 128, NPFP8)
        ee = np.nonzero(live_e & valid_e)[0]
        oh[ee * 128 + dstloc_e[ee]] = 1.0
        oh = oh.reshape(NCHUNK, 128, 128)
        ohT = np.zeros((NCHUNK, 128, 128), NPFP8)
        el = np.nonzero(live_e)[0]
        ohT[el >> 7, dstloc_e[el], el & 127] = 1.0
        edges.append(dict(
            src16=wrap_idx(src_e),
            oh=np.ascontiguousarray(oh.transpose(1, 0, 2)),
            ohT=np.ascontiguousarray(ohT.transpose(1, 0, 2)),
        ))

    # ---- weights / constants
    scale_q = 1.0 / math.sqrt(HD)
    Wq_s = np.ascontiguousarray((np.asarray(Wq) * scale_q).astype(NPBF16))
    Wk_s = np.ascontiguousarray(np.asarray(Wk).astype(NPBF16))
    Wv_s = np.ascontiguousarray(np.asarray(Wv).astype(NPBF16))
    bias3 = np.zeros((128, 3, D), np.float32)
    bias3[:, 0, :] = np.asarray(bq) * scale_q
    bias3[:, 1, :] = bk
    bias3[:, 2, :] = bv
    Wo_bf = np.ascontiguousarray(np.asarray(Wo).astype(NPBF16))

    ident = np.eye(128, dtype=np.float32).astype(NPBF16)
    g_rep = np.ascontiguousarray(
        np.broadcast_to(np.asarray(ln_g, np.float32), (128, D)))
    b_rep = np.ascontiguousarray(
        np.broadcast_to(np.asarray(ln_b, np.float32), (128, D)))

    in_maps = []
    for c in range(NCORES):
        rows = slice(c * NR, (c + 1) * NR)
        xTown = np.ascontiguousarray(x[rows].T.astype(NPBF16))
        xb = np.ascontiguousarray(x[rows] + np.asarray(bo, np.float32))
        m = dict(
            xTown=xTown, Wq=Wq_s, Wk=Wk_s, Wv=Wv_s,
            bias3=bias3, Wo=Wo_bf, xb=xb, g_rep=g_rep, b_rep=b_rep,
            ident=ident,
            **edges[c],
        )
        in_maps.append(m)
    zero_bias = not (np.any(bias3) or False)
    meta = dict(C_BLK=C_BLK, NCHUNK=NCHUNK, EP=EP, zero_bias=zero_bias)
    return in_maps, meta


def build_program(cfg, C_BLK, zero_bias=False):
    N, D, H, HD = cfg["N"], cfg["D"], cfg["H"], cfg["HD"]
    NR, NBLK, DC = cfg["NR"], cfg["NBLK"], cfg["DC"]
    NCHUNK = NBLK * C_BLK
    EP = NCHUNK * 128
    NT = N // 128
    NTO = NR // 128

    nc = bacc.Bacc(None, target_bir_lowering=False, debug=False,
                   num_devices=NCORES)

    xTown_in = nc.dram_tensor("xTown", [D, NR], BF16, kind="ExternalInput")
    Wq_in = nc.dram_tensor("Wq", [D, D], BF16, kind="ExternalInput")
    Wk_in = nc.dram_tensor("Wk", [D, D], BF16, kind="ExternalInput")
    Wv_in = nc.dram_tensor("Wv", [D, D], BF16, kind="ExternalInput")
    bias3_in = nc.dram_tensor("bias3", [128, 3, D], F32, kind="ExternalInput")
    Wo_in = nc.dram_tensor("Wo", [D, D], BF16, kind="ExternalInput")
    xb_in = nc.dram_tensor("xb", [NR, D], F32, kind="ExternalInput")
    g_rep_in = nc.dram_tensor("g_rep", [128, D], F32, kind="ExternalInput")
    b_rep_in = nc.dram_tensor("b_rep", [128, D], F32, kind="ExternalInput")
    ident_in = nc.dram_tensor("ident", [128, 128], BF16, kind="ExternalInput")
    src16_in = nc.dram_tensor("src16", [128, EP // 16], I16, kind="ExternalInput")
    oh_in = nc.dram_tensor("oh", [128, NCHUNK, 128], FP8, kind="ExternalInput")
    ohT_in = nc.dram_tensor("ohT", [128, NCHUNK, 128], FP8, kind="ExternalInput")

    out_ext = nc.dram_tensor("out", [NR, D], F32, kind="ExternalOutput")

    # AllGather destinations: Shared scratchpad for fast HBM-HBM collective
    hA = nc.dram_tensor("hAsh", [N, D], BF16, kind="Internal",
                        addr_space="Shared")
    hB = nc.dram_tensor("hBsh", [N, D], BF16, kind="Internal",
                        addr_space="Shared")
    kv_dram = nc.dram_tensor("kvsh", [N, 2 * D], BF16, kind="Internal",
                             addr_space="Shared")

    AG = [list(range(NCORES))]

    with tile.TileContext(nc) as tc:
        with (
            tc.tile_pool(name="res", bufs=1) as res,
            tc.tile_pool(name="dram", bufs=1, space="DRAM") as dram,
        ):
            ident_sb = res.tile([128, 128], BF16)
            nc.sync.dma_start(ident_sb[:], ident_in[:])
            ln72_sb = res.tile([128, 1], F32)
            nc.vector.memset(ln72_sb[:], float(np.log(0.9 * KAPPA)))
            src16_sb = res.tile([128, EP // 16], I16)
            nc.sync.dma_start(src16_sb[:], src16_in[:])
            escale_sb = res.tile([128, NCHUNK, H, 2], BF16)
            v01_sb = res.tile([128, NTO, D], BF16)
            h5_sb = res.tile([128, NTO, D], BF16)
            rdenom_sb = res.tile([128, NBLK, H], F32)

            q_dram = dram.tile([NR, D], BF16)
            kvshard = dram.tile([NR, 2 * D], BF16)
            shard = dram.tile([NR, D], BF16)

            # =========== P0: projections ===========
            with (
                tc.tile_pool(name="p0", bufs=3) as p0,
                tc.tile_pool(name="p0c", bufs=1) as p0c,
                tc.tile_pool(name="p0ps", bufs=4, space="PSUM") as p0ps,
            ):
                xTo_sb = p0c.tile([128, DC, NR], BF16)
                nc.sync.dma_start(
                    xTo_sb[:], xTown_in[:].rearrange("(c p) n -> p c n", p=128))
                W_sb = p0c.tile([128, 3, DC, D], BF16)
                for i, W in enumerate([Wq_in, Wk_in, Wv_in]):
                    nc.sync.dma_start(
                        W_sb[:, i, :, :],
                        W[:].rearrange("(c p) g -> p c g", p=128))
                bias_sb = p0c.tile([128, 3, D], F32)
                nc.sync.dma_start(bias_sb[:], bias3_in[:])

                def proj_tile(xsrc, toff, wi, stores, v01_t=None):
                    ps = p0ps.tile([128, D], F32, tag="pps")
                    for c in range(DC):
                        for j in range(2):
                            js = slice(j * 512, min((j + 1) * 512, D))
                            nc.tensor.matmul(
                                ps[:, js], xsrc[:, c, toff:toff + 128],
                                W_sb[:, wi, c, js],
                                start=(c == 0), stop=(c == DC - 1))
                    stg = p0.tile([128, D], BF16, tag="pstg")
                    if zero_bias:
                        nc.scalar.copy(stg[:], ps[:])
                    else:
                        nc.vector.tensor_tensor(
                            stg[:], ps[:], bias_sb[:, wi, :],
                            mybir.AluOpType.add)
                    for dest in stores:
                        nc.sync.dma_start(dest, stg[:])
                    if v01_t is not None:
                        nc.vector.tensor_scalar(
                            v01_sb[:, v01_t, :], stg[:], ALPHA, None,
                            mybir.AluOpType.mult)

                # kv for own shard first, so the AllGather launches early;
                # q projections overlap with the collective
                for t in range(NTO):
                    r = slice(t * 128, (t + 1) * 128)
                    proj_tile(xTo_sb, t * 128, 1, [kvshard[r, 0:D]])
                for t in range(NTO):
                    r = slice(t * 128, (t + 1) * 128)
                    proj_tile(xTo_sb, t * 128, 2, [kvshard[r, D:2 * D]],
                              v01_t=t)
                nc.gpsimd.collective_compute(
                    "AllGather", mybir.AluOpType.bypass,
                    replica_groups=AG,
                    ins=[kvshard[:].opt()], outs=[kv_dram[:].opt()])
                for t in range(NTO):
                    proj_tile(xTo_sb, t * 128, 0,
                              [q_dram[t * 128:(t + 1) * 128, :]])

            # =========== P1+P2: scores (step 0) + diffusion ===========
            with (
                tc.tile_pool(name="p2kv", bufs=2) as p2kv,
                tc.tile_pool(name="p2h", bufs=3) as p2h,
                tc.tile_pool(name="p2oh", bufs=3) as p2oh,
                tc.tile_pool(name="p2q", bufs=2) as p2q,
                tc.tile_pool(name="p2qb", bufs=2) as p2qb,
                tc.tile_pool(name="p2s", bufs=2) as p2s,
                tc.tile_pool(name="p2ps", bufs=2, space="PSUM") as p2ps,
                tc.tile_pool(name="p2psq", bufs=2, space="PSUM") as p2psq,
            ):
                hsrcs = [kv_dram, hB, hA, hB, hA]
                hdsts = [hB, hA, hB, hA, None]
                psm = None
                qblk = None
                for s in range(NSTEPS):
                    for g in range(NCHUNK // GCH):
                        q0 = g * GCH
                        ic = q0 * 8
                        gs = slice(q0, q0 + GCH)
                        oh = p2oh.tile([128, GCH, 128], FP8, tag="oh")
                        nc.sync.dma_start(oh[:], oh_in[:, gs, :])
                        if s == 0:
                            G = p2kv.tile([128, GCH, 2 * D], BF16, tag="Gkv")
                            nc.gpsimd.dma_gather(
                                G[:], kv_dram[:],
                                src16_sb[:, ic:ic + GCH * 8],
                                GCH * 128, GCH * 128, 2 * D)
                            ohT = p2q.tile([128, GCH, 128], FP8, tag="ohT")
                            nc.sync.dma_start(ohT[:], ohT_in[:, gs, :])
                            vG = G[:, :, D:2 * D]
                            # qsel[e,:] = q_blk[dstloc(e),:] via ohT matmul,
                            # per chunk; scores batched per group below
                            qb16 = p2s.tile([128, GCH, D], BF16, tag="qb16")
                            for gc in range(GCH):
                                q = q0 + gc
                                blk, ch = divmod(q, C_BLK)
                                if ch == 0:
                                    qblk = p2qb.tile([128, D], BF16,
                                                     tag="qblk")
                                    nc.sync.dma_start(
                                        qblk[:],
                                        q_dram[blk * 128:(blk + 1) * 128, :])
                                qps = p2psq.tile([128, D], F32, tag="qps")
                                for j in range(2):
                                    js = slice(j * 512, min((j + 1) * 512, D))
                                    nc.tensor.matmul(
                                        qps[:, js], ohT[:, gc, :],
                                        qblk[:, js], start=True, stop=True)
                                nc.scalar.copy(qb16[:, gc, :], qps[:])
                            # prod (in place over qb16), per-head tree reduce
                            qbh = qb16[:].rearrange(
                                "p c (h f) -> p c h f", h=H)
                            nc.vector.tensor_tensor(
                                qbh,
                                G[:, :, 0:D].rearrange(
                                    "p c (h f) -> p c h f", h=H),
                                qbh, mybir.AluOpType.mult)
                            w = HD // 2
                            while w >= 1:
                                nc.vector.tensor_tensor(
                                    qbh[:, :, :, 0:w], qbh[:, :, :, 0:w],
                                    qbh[:, :, :, w:2 * w],
                                    mybir.AluOpType.add)
                                w //= 2
                            nc.scalar.activation(
                                escale_sb[:, gs, :, :],
                                qbh[:, :, :, 0:1].to_broadcast(
                                    (128, GCH, H, 2)),
                                mybir.ActivationFunctionType.Exp,
                                bias=ln72_sb[:], scale=1.0)
                        else:
                            G = p2h.tile([128, GCH, D], BF16, tag="Gh")
                            nc.gpsimd.dma_gather(
                                G[:], hsrcs[s][:],
                                src16_sb[:, ic:ic + GCH * 8],
                                GCH * 128, GCH * 128, D)
                            vG = G[:, :, :]
                        nc.vector.tensor_tensor(
                            vG.rearrange(
                                "p c (h f2 two) -> p c h f2 two",
                                h=H, two=2),
                            vG.rearrange(
                                "p c (h f2 two) -> p c h f2 two",
                                h=H, two=2),
                            escale_sb[:, gs, :, :].unsqueeze(3)
                            .to_broadcast((128, GCH, H, HD // 2, 2)),
                            mybir.AluOpType.mult)
                        voff = D if s == 0 else 0
                        for gc in range(GCH):
                            q = q0 + gc
                            blk, ch = divmod(q, C_BLK)
                            if ch == 0:
                                psm = p2ps.tile([128, 832], F32, tag="psm")
                            for j in range(2):
                                js = slice(voff + j * 512,
                                           voff + min((j + 1) * 512, D))
                                os_ = slice(j * 512, min((j + 1) * 512, D))
                                nc.tensor.matmul(
                                    psm[:, os_], oh[:, gc, :], G[:, gc, js],
                                    start=(ch == 0), stop=(ch == C_BLK - 1))
                            if s == 0:
                                nc.tensor.matmul(
                                    psm[:, 768:768 + H], oh[:, gc, :],
                                    escale_sb[:, q, :, 0:1].rearrange(
                                        "p h one -> p (h one)"),
                                    start=(ch == 0), stop=(ch == C_BLK - 1),
                                    skip_group_check=True)
                            if ch == C_BLK - 1:
                                if s == 0:
                                    dn = p2s.tile([128, H], F32, tag="dn")
                                    nc.vector.tensor_scalar(
                                        dn[:], psm[:, 768:768 + H], 1e-9,
                                        None, mybir.AluOpType.max)
                                    dn2 = p2s.tile([128, H], F32, tag="dn2")
                                    nc.vector.reciprocal(dn2[:], dn[:])
                                    nc.vector.tensor_scalar(
                                        rdenom_sb[:, blk, :], dn2[:], 0.9,
                                        None, mybir.AluOpType.mult)
                                if s == NSTEPS - 1:
                                    stg_ap = h5_sb[:, blk, :]
                                else:
                                    stg = p2s.tile([128, D], BF16,
                                                   tag="hstg")
                                    stg_ap = stg[:]
                                for h in range(H):
                                    hs = slice(h * HD, (h + 1) * HD)
                                    nc.vector.scalar_tensor_tensor(
                                        stg_ap[:, hs], psm[:, hs],
                                        rdenom_sb[:, blk, h:h + 1],
                                        v01_sb[:, blk, hs],
                                        mybir.AluOpType.mult,
                                        mybir.AluOpType.add)
                                if s < NSTEPS - 1:
                                    nc.sync.dma_start(
                                        shard[blk * 128:(blk + 1) * 128, :],
                                        stg_ap)
                    if s < NSTEPS - 1:
                        nc.gpsimd.collective_compute(
                            "AllGather", mybir.AluOpType.bypass,
                            replica_groups=AG,
                            ins=[shard[:].opt()], outs=[hdsts[s][:].opt()])

            # =========== P3: output projection + LN ===========
            with (
                tc.tile_pool(name="p3", bufs=2) as p3,
                tc.tile_pool(name="p3c", bufs=1) as p3c,
                tc.tile_pool(name="p3ps", bufs=4, space="PSUM") as p3ps,
                tc.tile_pool(name="p3ps2", bufs=2, space="PSUM") as p3ps2,
            ):
                g_sb = p3c.tile([128, D], F32)
                nc.sync.dma_start(g_sb[:], g_rep_in[:])
                b_sb = p3c.tile([128, D], F32)
                nc.sync.dma_start(b_sb[:], b_rep_in[:])
                h5T_sb = p3c.tile([128, DC, NR], BF16)
                for t in range(NTO):
                    for c in range(DC):
                        tp = p3ps.tile([128, 128], BF16, tag="tp")
                        nc.tensor.transpose(
                            tp[:], h5_sb[:, t, c * 128:(c + 1) * 128],
                            ident_sb[:])
                        nc.vector.tensor_copy(
                            h5T_sb[:, c, t * 128:(t + 1) * 128], tp[:])
                Wo_sb = p3c.tile([128, DC, D], BF16)
                nc.sync.dma_start(
                    Wo_sb[:], Wo_in[:].rearrange("(c p) n -> p c n", p=128))
                for t in range(NTO):
                    yps = p3ps2.tile([128, D], F32, tag="yps")
                    for c in range(DC):
                        for j in range(2):
                            js = slice(j * 512, min((j + 1) * 512, D))
                            nc.tensor.matmul(
                                yps[:, js],
                                h5T_sb[:, c, t * 128:(t + 1) * 128],
                                Wo_sb[:, c, js],
                                start=(c == 0), stop=(c == DC - 1))
                    xb_sb = p3.tile([128, D], F32, tag="xb")
                    nc.sync.dma_start(xb_sb[:], xb_in[t * 128:(t + 1) * 128, :])
                    y_sb = p3.tile([128, D], F32, tag="y")
                    nc.vector.tensor_tensor(
                        y_sb[:], yps[:], xb_sb[:], mybir.AluOpType.add)
                    mu = p3.tile([128, 1], F32, tag="mu")
                    nc.vector.tensor_reduce(
                        mu[:], y_sb[:], mybir.AxisListType.X,
                        mybir.AluOpType.add)
                    negmu = p3.tile([128, 1], F32, tag="negmu")
                    nc.vector.tensor_scalar(
                        negmu[:], mu[:], -1.0 / D, None, mybir.AluOpType.mult)
                    sq = p3.tile([128, D], F32, tag="sq")
                    var = p3.tile([128, 1], F32, tag="var")
                    nc.scalar.activation(
                        sq[:], y_sb[:], mybir.ActivationFunctionType.Square,
                        bias=negmu[:], scale=1.0, accum_out=var[:])
                    vs = p3.tile([128, 1], F32, tag="vs")
                    nc.vector.tensor_scalar(
                        vs[:], var[:], 1.0 / D, LN_EPS,
                        mybir.AluOpType.mult, mybir.AluOpType.add)
                    std = p3.tile([128, 1], F32, tag="std")
                    nc.scalar.sqrt(std[:], vs[:])
                    rstd = p3.tile([128, 1], F32, tag="rstd")
                    nc.vector.reciprocal(rstd[:], std[:])
                    t1 = p3.tile([128, D], F32, tag="t1")
                    nc.vector.scalar_tensor_tensor(
                        t1[:], y_sb[:], negmu[:], g_sb[:],
                        mybir.AluOpType.add, mybir.AluOpType.mult)
                    outt = p3.tile([128, D], F32, tag="outt")
                    nc.vector.scalar_tensor_tensor(
                        outt[:], t1[:], rstd[:], b_sb[:],
                        mybir.AluOpType.mult, mybir.AluOpType.add)
                    nc.sync.dma_start(
                        out_ext[t * 128:(t + 1) * 128, :], outt[:])

    nc.compile()
    return nc


_PROG_CACHE = {}


def _get_program(cfg, C_BLK, zero_bias):
    key = (cfg["N"], cfg["E"], cfg["D"], cfg["H"], C_BLK, zero_bias)
    if key not in _PROG_CACHE:
        _PROG_CACHE[key] = build_program(cfg, C_BLK, zero_bias)
    return _PROG_CACHE[key]


def run(cfg, inputs, trace=False):
    in_maps, meta = host_prep(cfg, **inputs)
    nc = _get_program(cfg, meta["C_BLK"], meta["zero_bias"])
    res = run_bass_kernel_spmd(
        nc, in_maps, core_ids=list(range(NCORES)), trace=trace)
    N, D, NR = cfg["N"], cfg["D"], cfg["NR"]
    full = np.empty((N, D), np.float32)
    for r in range(NRANGE):
        full[r * NR:(r + 1) * NR] = res.results[r]["out"]
    return full.reshape(cfg["B"], cfg["S"], D), res


def kernel(**inputs):
    cfg = _cfg(B=2, S=4096, D=768, H=12, E=524288)
    out, _ = run(cfg, inputs)
    return out
